# revision 7
# baseline (speedup 1.0000x reference)
"""Trainium2 Bass kernel for nn_Attention_6468220748045.

Computes, per batch item: QKV projection -> per-head scaled attention with a
multiplicative positional bias w[i,j] = |i-j|/S -> softmax -> attn @ V ->
LayerNorm over the embedding dim.

Sharding: pure data-parallel over batch. B=128 splits as 16 batch items per
core across 8 NeuronCores; no collectives needed. Inputs are pre-laid-out on
host: x is passed transposed per batch ([B, E, S]) so both projection
orientations stream directly from SBUF, and the weights are passed transposed
([e_in, e_out]) to serve as matmul stationary operands.

Per-core kernel layout choices:
  - QT/KT projections: stationary = W.T tile [e_in,128 x e_out,128], moving =
    x.T for a PAIR of batches ([e_in,128 x 358]) -> fp32r runs at full rate
    (moving dim >= 256). Output orientation [e_out, s] is exactly what the
    scores matmul needs (contraction over head dim on the partition axis).
  - V projection: stationary = x.T tile, moving = Wv.T ([e_in,128 x 512]),
    giving V in natural [s, e] orientation for the PV matmul.
  - Scores: s.T[j, i] = (k_h).T.T @ (q_h).T in bf16, multiplied by the
    host-precomputed scale*w[j, i], exponentiated on ScalarE (no max
    subtraction: |scores| <= ~2, exp is safe).
  - Softmax denominator comes for free from the PV matmul: V is stored padded
    [s, H, 65] with a ones column, so out[:, 64] = sum_j p[j, i].
  - PV: stationary = p.T tile (bf16), moving = padded V tile; normalize by the
    reciprocal of the ones-column and write straight into the [s, e] output
    tile, which then gets LayerNorm'd (bn_stats/bn_aggr) and DMA'd out.
"""

import numpy as np

import concourse.bass as bass
import concourse.tile as tile
from concourse import bacc, mybir
from concourse.bass_utils import run_bass_kernel_spmd

# Problem constants (hardcoded per the self-contained-kernel contract).
B, S, E, H, D = 128, 179, 1024, 16, 64
NCORES = 8
BPC = B // NCORES          # batches per core = 16
NPAIR = BPC // 2           # batch pairs per core = 8
KT = E // 128              # contraction tiles over e_in = 8
MT = E // 128              # output tiles over e_out = 8
S0 = 128                   # first s-tile size
S1 = S - S0                # second s-tile size = 51
S_TILES = ((0, S0), (S0, S1))
LN_EPS = 1e-5
SCALE = float(E) ** -0.5

F32 = mybir.dt.float32
F32R = mybir.dt.float32r
BF16 = mybir.dt.bfloat16

AF = mybir.ActivationFunctionType
ALU = mybir.AluOpType


def _build_kernel(bpc: int = BPC) -> bass.Bass:
    npair = bpc // 2
    nc = bacc.Bacc()

    xT = nc.dram_tensor("xT", [bpc, E, S], BF16, kind="ExternalInput").ap()
    wqT = nc.dram_tensor("wqT", [E, E], BF16, kind="ExternalInput").ap()
    wkT = nc.dram_tensor("wkT", [E, E], BF16, kind="ExternalInput").ap()
    wvT = nc.dram_tensor("wvT", [E, E], BF16, kind="ExternalInput").ap()
    wsc = nc.dram_tensor("wsc", [S, S], F32, kind="ExternalInput").ap()
    gamma = nc.dram_tensor("gamma", [E], F32, kind="ExternalInput").ap()
    beta = nc.dram_tensor("beta", [E], F32, kind="ExternalInput").ap()
    out = nc.dram_tensor("out", [bpc, S, E], F32, kind="ExternalOutput").ap()

    with tile.TileContext(nc) as tc:
        _emit(tc, npair, out, xT, wqT, wkT, wvT, wsc, gamma, beta)
    nc.compile()
    return nc


def _emit(tc, npair, out, xT, wqT, wkT, wvT, wsc, gamma, beta):
    nc = tc.nc
    from contextlib import ExitStack

    with ExitStack() as ctx:
        singles = ctx.enter_context(tc.tile_pool(name="singles", bufs=1))
        xt_pool = ctx.enter_context(tc.tile_pool(name="xt", bufs=2))
        qk_pool = ctx.enter_context(tc.tile_pool(name="qk", bufs=2))
        v_pool = ctx.enter_context(tc.tile_pool(name="v", bufs=2))
        p_pool = ctx.enter_context(tc.tile_pool(name="p", bufs=4))
        o_pool = ctx.enter_context(tc.tile_pool(name="o", bufs=2))
        ln_pool = ctx.enter_context(tc.tile_pool(name="ln", bufs=4))
        r_pool = ctx.enter_context(tc.tile_pool(name="r", bufs=8))

        pp_qk = ctx.enter_context(tc.tile_pool(name="pp_qk", bufs=2, space="PSUM"))
        pp_v = ctx.enter_context(tc.tile_pool(name="pp_v", bufs=2, space="PSUM"))
        pp_s = ctx.enter_context(tc.tile_pool(name="pp_s", bufs=2, space="PSUM"))
        pp_o = ctx.enter_context(tc.tile_pool(name="pp_o", bufs=2, space="PSUM"))

        # --- resident tensors -------------------------------------------------
        # Weight tiles: [e_in partition, k-tile, e_out]
        w_sbs = []
        for name, wap in (("wq", wqT), ("wk", wkT), ("wv", wvT)):
            w_sb = singles.tile([128, KT, E], BF16, tag=f"w_{name}")
            src = wap.rearrange("(k p) e -> k p e", p=128)
            for k in range(KT):
                nc.sync.dma_start(out=w_sb[:, k], in_=src[k])
            w_sbs.append(w_sb)
        wq_sb, wk_sb, wv_sb = w_sbs

        # Positional bias (already includes softmax scale): [j partition, jt, i]
        wsc_sb = singles.tile([128, 2, S], F32, tag="wsc")
        nc.sync.dma_start(out=wsc_sb[:, 0], in_=wsc[0:S0])
        nc.sync.dma_start(out=wsc_sb[0:S1, 1], in_=wsc[S0:S])

        # gamma/beta broadcast to all partitions; eps scalar.
        gamma_b = singles.tile([128, E], F32, tag="gamma")
        beta_b = singles.tile([128, E], F32, tag="beta")
        nc.sync.dma_start(
            out=gamma_b,
            in_=bass.AP(tensor=gamma.tensor, offset=gamma.offset, ap=[[0, 128]] + gamma.ap),
        )
        nc.sync.dma_start(
            out=beta_b,
            in_=bass.AP(tensor=beta.tensor, offset=beta.offset, ap=[[0, 128]] + beta.ap),
        )
        eps_t = singles.tile([128, 1], F32, tag="eps")
        nc.vector.memset(eps_t, LN_EPS)

        xsrc = xT.rearrange("b (k p) s -> k p b s", p=128)  # [KT, 128, bpc, S]

        for pr in range(npair):
            # --- load x.T for this batch pair --------------------------------
            xt = xt_pool.tile([128, KT, 2, S], BF16)
            for k in range(KT):
                nc.sync.dma_start(out=xt[:, k], in_=xsrc[k, :, 2 * pr : 2 * pr + 2, :])

            # --- Q.T / K.T projections (both batches at once) ----------------
            # out[e_out, s2] with s2 = 2*S = 358 >= 256 -> fp32r full rate
            qt_sb = qk_pool.tile([128, MT, 2, S], BF16, tag="qt")
            kt_sb = qk_pool.tile([128, MT, 2, S], BF16, tag="kt")
            for w_sb, dst in ((wq_sb, qt_sb), (wk_sb, kt_sb)):
                for m in range(MT):
                    ps = pp_qk.tile([128, 2, S], F32)
                    for k in range(KT):
                        nc.tensor.matmul(
                            out=ps,
                            lhsT=w_sb[:, k, m * 128 : (m + 1) * 128],
                            rhs=xt[:, k],
                            start=(k == 0),
                            stop=(k == KT - 1),
                        )
                    nc.vector.tensor_copy(out=dst[:, m], in_=ps)

            for bi in range(2):
                b = 2 * pr + bi

                # --- V projection, natural [s, e] layout, padded ones col ----
                vpads = []
                for st, (ss, sn) in enumerate(S_TILES):
                    vp = v_pool.tile([128, H, D + 1], BF16, tag=f"vpad{st}")
                    nc.vector.memset(vp[:sn, :, D : D + 1], 1.0)
                    for n in range(2):
                        ps = pp_v.tile([128, 512], F32)
                        for k in range(KT):
                            nc.tensor.matmul(
                                out=ps[:sn],
                                lhsT=xt[:, k, bi, ss : ss + sn],
                                rhs=wv_sb[:, k, n * 512 : (n + 1) * 512],
                                start=(k == 0),
                                stop=(k == KT - 1),
                            )
                        nc.vector.tensor_copy(
                            out=vp[:sn, n * 8 : (n + 1) * 8, 0:D],
                            in_=ps[:sn].rearrange("p (h d) -> p h d", d=D),
                        )
                    vpads.append(vp)

                # --- attention -----------------------------------------------
                o_tiles = [
                    o_pool.tile([128, E], F32, tag=f"o{st}", name=f"o{st}_{b}")
                    for st, _ in enumerate(S_TILES)
                ]
                for h in range(H):
                    m, r0 = h // 2, (h % 2) * D
                    # scores.T[j, i] per j-tile, * (scale*w), exp -> bf16 p
                    p_tiles = []
                    for jt, (js, jn) in enumerate(S_TILES):
                        ps_s = pp_s.tile([128, S], F32, tag="s")
                        nc.tensor.matmul(
                            out=ps_s[:jn],
                            lhsT=kt_sb[r0 : r0 + D, m, bi, js : js + jn],
                            rhs=qt_sb[r0 : r0 + D, m, bi, :],
                            start=True,
                            stop=True,
                        )
                        nc.vector.tensor_mul(
                            out=ps_s[:jn], in0=ps_s[:jn], in1=wsc_sb[:jn, jt, :]
                        )
                        p_t = p_pool.tile([128, S], BF16, tag="p")
                        nc.scalar.activation(out=p_t[:jn], in_=ps_s[:jn], func=AF.Exp)
                        p_tiles.append(p_t)

                    # PV: out[i, 0:64] = sum_j p[j,i] v[j,:], out[i,64] = denom
                    for it, (is_, in_n) in enumerate(S_TILES):
                        ps_o = pp_o.tile([128, D + 1], F32, tag="po")
                        for jt, (js, jn) in enumerate(S_TILES):
                            nc.tensor.matmul(
                                out=ps_o[:in_n],
                                lhsT=p_tiles[jt][:jn, is_ : is_ + in_n],
                                rhs=vpads[jt][:jn, h],
                                start=(jt == 0),
                                stop=(jt == 1),
                            )
                        rec = r_pool.tile([128, 1], F32, tag="rec")
                        nc.vector.reciprocal(out=rec[:in_n], in_=ps_o[:in_n, D : D + 1])
                        nc.vector.tensor_scalar_mul(
                            out=o_tiles[it][:in_n, h * D : (h + 1) * D],
                            in0=ps_o[:in_n, 0:D],
                            scalar1=rec[:in_n],
                        )

                # --- LayerNorm over E + gamma/beta + store -------------------
                for it, (is_, in_n) in enumerate(S_TILES):
                    o_sb = o_tiles[it]
                    stats = ln_pool.tile([128, 2, 6], F32, tag="stats")
                    mv = ln_pool.tile([128, 2], F32, tag="mv")
                    nc.vector.bn_stats(out=stats[:in_n, 0], in_=o_sb[:in_n, 0:512])
                    nc.vector.bn_stats(out=stats[:in_n, 1], in_=o_sb[:in_n, 512:E])
                    nc.vector.bn_aggr(out=mv[:in_n], in_=stats[:in_n])
                    rstd = r_pool.tile([128, 1], F32, tag="rstd")
                    nc.scalar.activation(
                        out=rstd[:in_n], in_=mv[:in_n, 1:2], func=AF.Sqrt, bias=eps_t[:in_n]
                    )
                    rrstd = r_pool.tile([128, 1], F32, tag="rrstd")
                    nc.vector.reciprocal(out=rrstd[:in_n], in_=rstd[:in_n])
                    nc.vector.tensor_scalar(
                        out=o_sb[:in_n],
                        in0=o_sb[:in_n],
                        scalar1=mv[:in_n, 0:1],
                        scalar2=rrstd[:in_n],
                        op0=ALU.subtract,
                        op1=ALU.mult,
                    )
                    nc.vector.tensor_mul(out=o_sb[:in_n], in0=o_sb[:in_n], in1=gamma_b[:in_n])
                    nc.vector.tensor_add(out=o_sb[:in_n], in0=o_sb[:in_n], in1=beta_b[:in_n])
                    nc.sync.dma_start(out=out[b, is_ : is_ + in_n], in_=o_sb[:in_n])


_NC_CACHE: dict[int, bass.Bass] = {}


def _get_nc(bpc: int = BPC) -> bass.Bass:
    if bpc not in _NC_CACHE:
        _NC_CACHE[bpc] = _build_kernel(bpc)
    return _NC_CACHE[bpc]


def _host_inputs(x, Wq, Wk, Wv, gamma, beta):
    import ml_dtypes

    bf16 = ml_dtypes.bfloat16
    x = np.asarray(x, dtype=np.float32)
    xT = np.ascontiguousarray(x.transpose(0, 2, 1)).astype(bf16)  # [B, E, S]
    idx = np.arange(S, dtype=np.float32)
    wsc = (np.abs(idx[None, :] - idx[:, None]) / S * SCALE).astype(np.float32)
    common = {
        "wqT": np.ascontiguousarray(np.asarray(Wq, np.float32).T).astype(bf16),
        "wkT": np.ascontiguousarray(np.asarray(Wk, np.float32).T).astype(bf16),
        "wvT": np.ascontiguousarray(np.asarray(Wv, np.float32).T).astype(bf16),
        "wsc": wsc,
        "gamma": np.asarray(gamma, np.float32),
        "beta": np.asarray(beta, np.float32),
    }
    return xT, common


def run(inputs: dict, trace: bool = False, trace_dir: str | None = None):
    """Run the SPMD kernel on 8 cores. Returns (full_output, exec_time_ns)."""
    xT, common = _host_inputs(**inputs)
    in_maps = [
        {**common, "xT": np.ascontiguousarray(xT[c * BPC : (c + 1) * BPC])}
        for c in range(NCORES)
    ]
    nc = _get_nc()
    res = run_bass_kernel_spmd(
        nc, in_maps, core_ids=list(range(NCORES)), trace=trace, tmpdir=trace_dir
    )
    full = np.concatenate([res.results[c]["out"] for c in range(NCORES)], axis=0)
    return full.astype(np.float32), res.exec_time_ns


def kernel(x, Wq, Wk, Wv, gamma, beta):
    full, _ = run(dict(x=x, Wq=Wq, Wk=Wk, Wv=Wv, gamma=gamma, beta=beta))
    return full


# revision 17
# speedup vs baseline: 1.2550x; 1.2550x over previous
"""Trainium2 Bass kernel for nn_Attention_6468220748045.

Computes, per batch item: QKV projection -> per-head scaled attention with a
multiplicative positional bias w[i,j] = |i-j|/S -> softmax -> attn @ V ->
LayerNorm over the embedding dim.

Sharding: pure data-parallel over batch. B=128 splits as 16 batch items per
core across 8 NeuronCores; no collectives needed. Inputs are pre-laid-out on
host: x is passed transposed per batch ([B, E, S]) so both projection
orientations stream directly from SBUF, and the weights are passed transposed
([e_in, e_out]) to serve as matmul stationary operands.

Per-core kernel layout choices:
  - QT/KT projections: stationary = W.T tile [e_in,128 x e_out,128], moving =
    x.T for a PAIR of batches ([e_in,128 x 358]) -> fp32r runs at full rate
    (moving dim >= 256). Output orientation [e_out, s] is exactly what the
    scores matmul needs (contraction over head dim on the partition axis).
  - V projection: stationary = x.T tile, moving = Wv.T ([e_in,128 x 512]),
    giving V in natural [s, e] orientation for the PV matmul.
  - Scores: s.T[j, i] = (k_h).T.T @ (q_h).T in bf16, multiplied by the
    host-precomputed scale*w[j, i], exponentiated on ScalarE (no max
    subtraction: |scores| <= ~2, exp is safe).
  - Softmax denominator comes for free from the PV matmul: V is stored padded
    [s, H, 65] with a ones column, so out[:, 64] = sum_j p[j, i].
  - PV: stationary = p.T tile (bf16), moving = padded V tile; normalize by the
    reciprocal of the ones-column and write straight into the [s, e] output
    tile, which then gets LayerNorm'd (bn_stats/bn_aggr) and DMA'd out.
"""

import numpy as np

import concourse.bass as bass
import concourse.tile as tile
from concourse import bacc, mybir
from concourse.bass_utils import run_bass_kernel_spmd

# Problem constants (hardcoded per the self-contained-kernel contract).
B, S, E, H, D = 128, 179, 1024, 16, 64
NCORES = 8
BPC = B // NCORES          # batches per core = 16
NPAIR = BPC // 2           # batch pairs per core = 8
KT = E // 128              # contraction tiles over e_in = 8
MT = E // 128              # output tiles over e_out = 8
S0 = 128                   # first s-tile size
S1 = S - S0                # second s-tile size = 51
S_TILES = ((0, S0), (S0, S1))
LN_EPS = 1e-5
SCALE = float(E) ** -0.5

F32 = mybir.dt.float32
F32R = mybir.dt.float32r
BF16 = mybir.dt.bfloat16

AF = mybir.ActivationFunctionType
ALU = mybir.AluOpType


def _build_kernel(bpc: int = BPC, apply_gb: bool = True) -> bass.Bass:
    npair = bpc // 2
    nc = bacc.Bacc()

    xT = nc.dram_tensor("xT", [bpc, E, S], BF16, kind="ExternalInput").ap()
    wqT = nc.dram_tensor("wqT", [E, E], BF16, kind="ExternalInput").ap()
    wkT = nc.dram_tensor("wkT", [E, E], BF16, kind="ExternalInput").ap()
    wvT = nc.dram_tensor("wvT", [E, E], BF16, kind="ExternalInput").ap()
    wsc = nc.dram_tensor("wsc", [S, S], F32, kind="ExternalInput").ap()
    gamma = nc.dram_tensor("gamma", [E], F32, kind="ExternalInput").ap()
    beta = nc.dram_tensor("beta", [E], F32, kind="ExternalInput").ap()
    out = nc.dram_tensor("out", [bpc, S, E], F32, kind="ExternalOutput").ap()

    with tile.TileContext(nc) as tc:
        _emit(tc, npair, out, xT, wqT, wkT, wvT, wsc, gamma, beta, apply_gb)
    nc.compile()
    return nc


def _emit(tc, npair, out, xT, wqT, wkT, wvT, wsc, gamma, beta, apply_gb):
    nc = tc.nc
    from contextlib import ExitStack

    with ExitStack() as ctx:
        singles = ctx.enter_context(tc.tile_pool(name="singles", bufs=1))
        xt_pool = ctx.enter_context(tc.tile_pool(name="xt", bufs=2))
        qk_pool = ctx.enter_context(tc.tile_pool(name="qk", bufs=2))
        v_pool = ctx.enter_context(tc.tile_pool(name="v", bufs=4))
        p_pool = ctx.enter_context(tc.tile_pool(name="p", bufs=4))
        o_pool = ctx.enter_context(tc.tile_pool(name="o", bufs=3))
        ln_pool = ctx.enter_context(tc.tile_pool(name="ln", bufs=4))
        r_pool = ctx.enter_context(tc.tile_pool(name="r", bufs=8))

        pp_qk = ctx.enter_context(tc.tile_pool(name="pp_qk", bufs=2, space="PSUM"))
        pp_v = ctx.enter_context(tc.tile_pool(name="pp_v", bufs=2, space="PSUM"))
        pp_s = ctx.enter_context(tc.tile_pool(name="pp_s", bufs=2, space="PSUM"))
        pp_o = ctx.enter_context(tc.tile_pool(name="pp_o", bufs=2, space="PSUM"))

        # --- resident tensors -------------------------------------------------
        # Weight tiles: [e_in partition, k-tile, e_out]
        w_sbs = []
        for name, wap in (("wq", wqT), ("wk", wkT), ("wv", wvT)):
            w_sb = singles.tile([128, KT, E], BF16, tag=f"w_{name}")
            src = wap.rearrange("(k p) e -> k p e", p=128)
            for k in range(KT):
                nc.sync.dma_start(out=w_sb[:, k], in_=src[k])
            w_sbs.append(w_sb)
        wq_sb, wk_sb, wv_sb = w_sbs

        # Positional bias (already includes softmax scale): [j partition, jt, i]
        wsc_sb = singles.tile([128, 2, S], F32, tag="wsc")
        nc.vector.memset(wsc_sb[:, 1], 0.0)
        nc.sync.dma_start(out=wsc_sb[:, 0], in_=wsc[0:S0])
        nc.sync.dma_start(out=wsc_sb[0:S1, 1], in_=wsc[S0:S])

        # gamma/beta broadcast to all partitions (skipped when the caller
        # verified they are identity); eps scalar.
        if apply_gb:
            gamma_b = singles.tile([128, E], F32, tag="gamma")
            beta_b = singles.tile([128, E], F32, tag="beta")
            nc.sync.dma_start(
                out=gamma_b,
                in_=bass.AP(tensor=gamma.tensor, offset=gamma.offset, ap=[[0, 128]] + gamma.ap),
            )
            nc.sync.dma_start(
                out=beta_b,
                in_=bass.AP(tensor=beta.tensor, offset=beta.offset, ap=[[0, 128]] + beta.ap),
            )
        eps_t = singles.tile([128, 1], F32, tag="eps")
        nc.vector.memset(eps_t, LN_EPS)

        xsrc = xT.rearrange("b (k p) s -> k p b s", p=128)  # [KT, 128, bpc, S]

        # Per-pair SBUF products handed from the projection stage to the
        # attention stage (software pipeline, depth 1).
        stage: dict = {}

        def proj_gen(pr):
            """QKV projections for batch pair `pr`; yields after each PE chunk
            (~8 matmuls) so attention of pair pr-1 can be interleaved."""
            xt = xt_pool.tile([128, KT, 2, S], BF16, tag="xt", name=f"xt_{pr}")
            for k in range(KT):
                nc.sync.dma_start(out=xt[:, k], in_=xsrc[k, :, 2 * pr : 2 * pr + 2, :])

            # Q.T / K.T: out[e_out, s2], s2 = 2*S = 358 (both batches at once)
            qt_sb = qk_pool.tile([128, MT, 2, S], BF16, tag="qt", name=f"qt_{pr}")
            kt_sb = qk_pool.tile([128, MT, 2, S], BF16, tag="kt", name=f"kt_{pr}")
            for w_sb, dst in ((wq_sb, qt_sb), (wk_sb, kt_sb)):
                for m in range(MT):
                    ps = pp_qk.tile([128, 2, S], F32, tag="qk", name=f"psqk_{pr}_{m}")
                    for k in range(KT):
                        nc.tensor.matmul(
                            out=ps,
                            lhsT=w_sb[:, k, m * 128 : (m + 1) * 128],
                            rhs=xt[:, k],
                            start=(k == 0),
                            stop=(k == KT - 1),
                        )
                    nc.vector.tensor_copy(out=dst[:, m], in_=ps)
                    yield

            # V: natural [s, e] layout with a ones column appended per head
            vpads_by_b = []
            for bi in range(2):
                vpads = []
                for st, (ss, sn) in enumerate(S_TILES):
                    vp = v_pool.tile(
                        [128, H, D + 1], BF16, tag=f"vpad{st}", name=f"vp{st}_{pr}_{bi}"
                    )
                    nc.vector.memset(vp[:sn, :, D : D + 1], 1.0)
                    for n in range(2):
                        ps = pp_v.tile([128, 512], F32, tag="v", name=f"psv_{pr}_{bi}_{st}_{n}")
                        for k in range(KT):
                            nc.tensor.matmul(
                                out=ps[:sn],
                                lhsT=xt[:, k, bi, ss : ss + sn],
                                rhs=wv_sb[:, k, n * 512 : (n + 1) * 512],
                                start=(k == 0),
                                stop=(k == KT - 1),
                            )
                        nc.vector.tensor_copy(
                            out=vp[:sn, n * 8 : (n + 1) * 8, 0:D],
                            in_=ps[:sn].rearrange("p (h d) -> p h d", d=D),
                        )
                        yield
                    vpads.append(vp)
                vpads_by_b.append(vpads)
            stage[pr] = (qt_sb, kt_sb, vpads_by_b)

        def attn_gen(pr):
            """Attention + LayerNorm for both batches of pair `pr`; yields per
            head so pair pr+1 projection matmuls can fill PE idle gaps."""
            qt_sb, kt_sb, vpads_by_b = stage.pop(pr)
            o_by_b = []
            for bi in range(2):
                b = 2 * pr + bi
                vpads = vpads_by_b[bi]
                o_tiles = [
                    o_pool.tile([128, E], F32, tag=f"o{st}", name=f"o{st}_{b}")
                    for st, _ in enumerate(S_TILES)
                ]
                o_by_b.append(o_tiles)
                ps_o4 = None
                for h in range(H):
                    m, r0 = h // 2, (h % 2) * D
                    # scores.T[j, i], both j-tiles in one psum bank -> one
                    # w-mul and one exp per head. Rows 51:128 of the jt=1
                    # slice hold stale psum garbage that is never read
                    # downstream (PV slices [:jn]).
                    ps_s = pp_s.tile([128, 2, S], F32, tag="s", name=f"pss_{b}_{h}")
                    nc.vector.memset(ps_s[:, 1], 0.0)
                    for jt, (js, jn) in enumerate(S_TILES):
                        nc.tensor.matmul(
                            out=ps_s[:jn, jt],
                            lhsT=kt_sb[r0 : r0 + D, m, bi, js : js + jn],
                            rhs=qt_sb[r0 : r0 + D, m, bi, :],
                            start=True,
                            stop=True,
                        )
                    nc.vector.tensor_mul(out=ps_s, in0=ps_s, in1=wsc_sb)
                    p_t = p_pool.tile([128, 2, S], BF16, tag="p", name=f"p_{b}_{h}")
                    nc.scalar.activation(out=p_t, in_=ps_s, func=AF.Exp)

                    # PV: 4 heads share a psum bank: [i, 4, 65] where col 64 of
                    # each head is the softmax denominator (ones column in V).
                    hc = h % 4
                    if hc == 0:
                        ps_o4 = [
                            pp_o.tile([128, 4, D + 1], F32, tag="po", name=f"pso_{b}_{h}_{it}")
                            for it, _ in enumerate(S_TILES)
                        ]
                    for it, (is_, in_n) in enumerate(S_TILES):
                        for jt, (js, jn) in enumerate(S_TILES):
                            nc.tensor.matmul(
                                out=ps_o4[it][:in_n, hc],
                                lhsT=p_t[:jn, jt, is_ : is_ + in_n],
                                rhs=vpads[jt][:jn, h],
                                start=(jt == 0),
                                stop=(jt == 1),
                            )
                    if hc == 3:
                        # Batched normalize for the 4-head group: one
                        # reciprocal of the 4 denominators, one broadcast
                        # multiply writing [i, 4*64] of the output tile.
                        g0 = (h - 3) * D
                        for it, (is_, in_n) in enumerate(S_TILES):
                            rec = r_pool.tile([128, 4], F32, tag="rec4", name=f"rc_{b}_{h}_{it}")
                            nc.vector.reciprocal(
                                out=rec[:in_n], in_=ps_o4[it][:in_n, :, D]
                            )
                            for c in range(4):
                                nc.vector.tensor_scalar_mul(
                                    out=o_tiles[it][:in_n, g0 + c * D : g0 + (c + 1) * D],
                                    in0=ps_o4[it][:in_n, c, 0:D],
                                    scalar1=rec[:in_n, c : c + 1],
                                )
                    yield

            # LayerNorm for both batches last: keeps the ACT table warm (all
            # Exp during attention, then all Sqrt).
            for bi in range(2):
                b = 2 * pr + bi
                for it, (is_, in_n) in enumerate(S_TILES):
                    o_sb = o_by_b[bi][it]
                    stats = ln_pool.tile([128, 2, 6], F32, tag="stats", name=f"st_{b}_{it}")
                    mv = ln_pool.tile([128, 2], F32, tag="mv", name=f"mv_{b}_{it}")
                    nc.vector.bn_stats(out=stats[:in_n, 0], in_=o_sb[:in_n, 0:512])
                    nc.vector.bn_stats(out=stats[:in_n, 1], in_=o_sb[:in_n, 512:E])
                    nc.vector.bn_aggr(out=mv[:in_n], in_=stats[:in_n])
                    rstd = r_pool.tile([128, 1], F32, tag="rstd", name=f"rs_{b}_{it}")
                    nc.scalar.activation(
                        out=rstd[:in_n], in_=mv[:in_n, 1:2], func=AF.Sqrt, bias=eps_t[:in_n]
                    )
                    rrstd = r_pool.tile([128, 1], F32, tag="rrstd", name=f"rr_{b}_{it}")
                    nc.vector.reciprocal(out=rrstd[:in_n], in_=rstd[:in_n])
                    nc.vector.tensor_scalar(
                        out=o_sb[:in_n],
                        in0=o_sb[:in_n],
                        scalar1=mv[:in_n, 0:1],
                        scalar2=rrstd[:in_n],
                        op0=ALU.subtract,
                        op1=ALU.mult,
                    )
                    if apply_gb:
                        nc.vector.tensor_mul(out=o_sb[:in_n], in0=o_sb[:in_n], in1=gamma_b[:in_n])
                        nc.vector.tensor_add(out=o_sb[:in_n], in0=o_sb[:in_n], in1=beta_b[:in_n])
                    nc.sync.dma_start(out=out[b, is_ : is_ + in_n], in_=o_sb[:in_n])
                yield

        # Software pipeline: attention(p) interleaved with projections(p+1) so
        # the PE instruction stream has no long idle stretches (HAM stays warm).
        N_PROJ_CHUNKS = 2 * MT + 8   # 24 yields per proj_gen
        N_ATTN_CHUNKS = 2 * H + 2    # 34 yields per attn_gen
        for _ in proj_gen(0):
            pass
        for p in range(npair):
            ag = attn_gen(p)
            pg = proj_gen(p + 1) if p + 1 < npair else None
            acc = 0
            for _ in ag:
                if pg is not None:
                    acc += N_PROJ_CHUNKS
                    while acc >= N_ATTN_CHUNKS:
                        next(pg, None)
                        acc -= N_ATTN_CHUNKS
            if pg is not None:
                for _ in pg:
                    pass


_NC_CACHE: dict = {}


def _get_nc(bpc: int = BPC, apply_gb: bool = True) -> bass.Bass:
    key = (bpc, apply_gb)
    if key not in _NC_CACHE:
        _NC_CACHE[key] = _build_kernel(bpc, apply_gb)
    return _NC_CACHE[key]


def _host_inputs(x, Wq, Wk, Wv, gamma, beta):
    import ml_dtypes

    bf16 = ml_dtypes.bfloat16
    x = np.asarray(x, dtype=np.float32)
    xT = np.ascontiguousarray(x.transpose(0, 2, 1)).astype(bf16)  # [B, E, S]
    idx = np.arange(S, dtype=np.float32)
    wsc = (np.abs(idx[None, :] - idx[:, None]) / S * SCALE).astype(np.float32)
    common = {
        "wqT": np.ascontiguousarray(np.asarray(Wq, np.float32).T).astype(bf16),
        "wkT": np.ascontiguousarray(np.asarray(Wk, np.float32).T).astype(bf16),
        "wvT": np.ascontiguousarray(np.asarray(Wv, np.float32).T).astype(bf16),
        "wsc": wsc,
        "gamma": np.asarray(gamma, np.float32),
        "beta": np.asarray(beta, np.float32),
    }
    return xT, common


def run(inputs: dict, trace: bool = False, trace_dir: str | None = None):
    """Run the SPMD kernel on 8 cores. Returns (full_output, exec_time_ns)."""
    xT, common = _host_inputs(**inputs)
    in_maps = [
        {**common, "xT": np.ascontiguousarray(xT[c * BPC : (c + 1) * BPC])}
        for c in range(NCORES)
    ]
    apply_gb = not (
        np.all(np.asarray(inputs["gamma"]) == 1.0)
        and np.all(np.asarray(inputs["beta"]) == 0.0)
    )
    nc = _get_nc(BPC, apply_gb)
    res = run_bass_kernel_spmd(
        nc, in_maps, core_ids=list(range(NCORES)), trace=trace, tmpdir=trace_dir
    )
    full = np.concatenate([res.results[c]["out"] for c in range(NCORES)], axis=0)
    return full.astype(np.float32), res.exec_time_ns


def kernel(x, Wq, Wk, Wv, gamma, beta):
    full, _ = run(dict(x=x, Wq=Wq, Wk=Wk, Wv=Wv, gamma=gamma, beta=beta))
    return full


# revision 22
# speedup vs baseline: 1.2691x; 1.0113x over previous
"""Trainium2 Bass kernel for nn_Attention_6468220748045.

Computes, per batch item: QKV projection -> per-head scaled attention with a
multiplicative positional bias w[i,j] = |i-j|/S -> softmax -> attn @ V ->
LayerNorm over the embedding dim.

Sharding: pure data-parallel over batch. B=128 splits as 16 batch items per
core across 8 NeuronCores; no collectives needed. Inputs are pre-laid-out on
host: x is passed transposed per batch ([B, E, S]) so both projection
orientations stream directly from SBUF, and the weights are passed transposed
([e_in, e_out]) to serve as matmul stationary operands.

Per-core kernel layout choices:
  - QT/KT projections: stationary = W.T tile [e_in,128 x e_out,128], moving =
    x.T for a PAIR of batches ([e_in,128 x 358]) -> fp32r runs at full rate
    (moving dim >= 256). Output orientation [e_out, s] is exactly what the
    scores matmul needs (contraction over head dim on the partition axis).
  - V projection: stationary = x.T tile, moving = Wv.T ([e_in,128 x 512]),
    giving V in natural [s, e] orientation for the PV matmul.
  - Scores: s.T[j, i] = (k_h).T.T @ (q_h).T in bf16, multiplied by the
    host-precomputed scale*w[j, i], exponentiated on ScalarE (no max
    subtraction: |scores| <= ~2, exp is safe).
  - Softmax denominator comes for free from the PV matmul: V is stored padded
    [s, H, 65] with a ones column, so out[:, 64] = sum_j p[j, i].
  - PV: stationary = p.T tile (bf16), moving = padded V tile; normalize by the
    reciprocal of the ones-column and write straight into the [s, e] output
    tile, which then gets LayerNorm'd (bn_stats/bn_aggr) and DMA'd out.
"""

import numpy as np

import concourse.bass as bass
import concourse.tile as tile
from concourse import bacc, mybir
from concourse.bass_utils import run_bass_kernel_spmd

# Problem constants (hardcoded per the self-contained-kernel contract).
B, S, E, H, D = 128, 179, 1024, 16, 64
NCORES = 8
BPC = B // NCORES          # batches per core = 16
NPAIR = BPC // 2           # batch pairs per core = 8
KT = E // 128              # contraction tiles over e_in = 8
MT = E // 128              # output tiles over e_out = 8
S0 = 128                   # first s-tile size
S1 = S - S0                # second s-tile size = 51
S_TILES = ((0, S0), (S0, S1))
LN_EPS = 1e-5
SCALE = float(E) ** -0.5

F32 = mybir.dt.float32
F32R = mybir.dt.float32r
BF16 = mybir.dt.bfloat16

AF = mybir.ActivationFunctionType
ALU = mybir.AluOpType


def _build_kernel(bpc: int = BPC, apply_gb: bool = True) -> bass.Bass:
    npair = bpc // 2
    nc = bacc.Bacc()

    xT = nc.dram_tensor("xT", [bpc, E, S], BF16, kind="ExternalInput").ap()
    wqT = nc.dram_tensor("wqT", [E, E], BF16, kind="ExternalInput").ap()
    wkT = nc.dram_tensor("wkT", [E, E], BF16, kind="ExternalInput").ap()
    wvT = nc.dram_tensor("wvT", [E, E], BF16, kind="ExternalInput").ap()
    wsc = nc.dram_tensor("wsc", [S, S], F32, kind="ExternalInput").ap()
    gamma = nc.dram_tensor("gamma", [E], F32, kind="ExternalInput").ap()
    beta = nc.dram_tensor("beta", [E], F32, kind="ExternalInput").ap()
    out = nc.dram_tensor("out", [bpc, S, E], F32, kind="ExternalOutput").ap()

    with tile.TileContext(nc) as tc:
        _emit(tc, npair, out, xT, wqT, wkT, wvT, wsc, gamma, beta, apply_gb)
    nc.compile()
    return nc


def _emit(tc, npair, out, xT, wqT, wkT, wvT, wsc, gamma, beta, apply_gb):
    nc = tc.nc
    from contextlib import ExitStack

    with ExitStack() as ctx:
        singles = ctx.enter_context(tc.tile_pool(name="singles", bufs=1))
        xt_pool = ctx.enter_context(tc.tile_pool(name="xt", bufs=3))
        qk_pool = ctx.enter_context(tc.tile_pool(name="qk", bufs=3))
        v_pool = ctx.enter_context(tc.tile_pool(name="v", bufs=6))
        p_pool = ctx.enter_context(tc.tile_pool(name="p", bufs=4))
        o_pool = ctx.enter_context(tc.tile_pool(name="o", bufs=3))
        ln_pool = ctx.enter_context(tc.tile_pool(name="ln", bufs=4))
        r_pool = ctx.enter_context(tc.tile_pool(name="r", bufs=8))

        pp_qk = ctx.enter_context(tc.tile_pool(name="pp_qk", bufs=2, space="PSUM"))
        pp_v = ctx.enter_context(tc.tile_pool(name="pp_v", bufs=2, space="PSUM"))
        pp_s = ctx.enter_context(tc.tile_pool(name="pp_s", bufs=2, space="PSUM"))
        pp_o = ctx.enter_context(tc.tile_pool(name="pp_o", bufs=2, space="PSUM"))

        # --- resident tensors -------------------------------------------------
        # Weight tiles: [e_in partition, k-tile, e_out]. DMA order matters for
        # startup latency: wq first, then pair-0's x.T, then wk/wv — the first
        # Q.T matmuls only need wq + x.T.
        xsrc = xT.rearrange("b (k p) s -> k p b s", p=128)  # [KT, 128, bpc, S]
        w_sbs = []
        for name, wap in (("wq", wqT), ("wk", wkT), ("wv", wvT)):
            w_sb = singles.tile([128, KT, E], BF16, tag=f"w_{name}")
            w_sbs.append(w_sb)
        wq_sb, wk_sb, wv_sb = w_sbs
        xt0 = xt_pool.tile([128, KT, 2, S], BF16, tag="xt", name="xt_0")
        for w_sb, wap in ((wq_sb, wqT),):
            src = wap.rearrange("(k p) e -> k p e", p=128)
            for k in range(KT):
                nc.sync.dma_start(out=w_sb[:, k], in_=src[k])
        for k in range(KT):
            nc.sync.dma_start(out=xt0[:, k], in_=xsrc[k, :, 0:2, :])
        for w_sb, wap in ((wk_sb, wkT), (wv_sb, wvT)):
            src = wap.rearrange("(k p) e -> k p e", p=128)
            for k in range(KT):
                nc.sync.dma_start(out=w_sb[:, k], in_=src[k])

        # Positional bias (already includes softmax scale): [j partition, jt, i]
        wsc_sb = singles.tile([128, 2, S], F32, tag="wsc")
        nc.vector.memset(wsc_sb[:, 1], 0.0)
        nc.sync.dma_start(out=wsc_sb[:, 0], in_=wsc[0:S0])
        nc.sync.dma_start(out=wsc_sb[0:S1, 1], in_=wsc[S0:S])

        # gamma/beta broadcast to all partitions (skipped when the caller
        # verified they are identity); eps scalar.
        if apply_gb:
            gamma_b = singles.tile([128, E], F32, tag="gamma")
            beta_b = singles.tile([128, E], F32, tag="beta")
            nc.sync.dma_start(
                out=gamma_b,
                in_=bass.AP(tensor=gamma.tensor, offset=gamma.offset, ap=[[0, 128]] + gamma.ap),
            )
            nc.sync.dma_start(
                out=beta_b,
                in_=bass.AP(tensor=beta.tensor, offset=beta.offset, ap=[[0, 128]] + beta.ap),
            )
        eps_t = singles.tile([128, 1], F32, tag="eps")
        nc.vector.memset(eps_t, LN_EPS)

        # Per-pair SBUF products handed from the projection stage to the
        # attention stage (software pipeline).
        stage: dict = {}

        def proj_gen(pr):
            """QKV projections for batch pair `pr`; yields after each PE chunk
            (~8 matmuls) so attention of pair pr-1 can be interleaved."""
            if pr == 0:
                xt = xt0
            else:
                xt = xt_pool.tile([128, KT, 2, S], BF16, tag="xt", name=f"xt_{pr}")
                for k in range(KT):
                    nc.sync.dma_start(
                        out=xt[:, k], in_=xsrc[k, :, 2 * pr : 2 * pr + 2, :]
                    )

            # Q.T / K.T: out[e_out, s2], s2 = 2*S = 358 (both batches at once)
            qt_sb = qk_pool.tile([128, MT, 2, S], BF16, tag="qt", name=f"qt_{pr}")
            kt_sb = qk_pool.tile([128, MT, 2, S], BF16, tag="kt", name=f"kt_{pr}")
            for w_sb, dst in ((wq_sb, qt_sb), (wk_sb, kt_sb)):
                for m in range(MT):
                    ps = pp_qk.tile([128, 2, S], F32, tag="qk", name=f"psqk_{pr}_{m}")
                    for k in range(KT):
                        nc.tensor.matmul(
                            out=ps,
                            lhsT=w_sb[:, k, m * 128 : (m + 1) * 128],
                            rhs=xt[:, k],
                            start=(k == 0),
                            stop=(k == KT - 1),
                        )
                    nc.vector.tensor_copy(out=dst[:, m], in_=ps)
                    yield

            # V: natural [s, e] layout with a ones column appended per head
            vpads_by_b = []
            for bi in range(2):
                vpads = []
                for st, (ss, sn) in enumerate(S_TILES):
                    vp = v_pool.tile(
                        [128, H, D + 1], BF16, tag=f"vpad{st}", name=f"vp{st}_{pr}_{bi}"
                    )
                    nc.vector.memset(vp[:sn, :, D : D + 1], 1.0)
                    for n in range(2):
                        ps = pp_v.tile([128, 512], F32, tag="v", name=f"psv_{pr}_{bi}_{st}_{n}")
                        for k in range(KT):
                            nc.tensor.matmul(
                                out=ps[:sn],
                                lhsT=xt[:, k, bi, ss : ss + sn],
                                rhs=wv_sb[:, k, n * 512 : (n + 1) * 512],
                                start=(k == 0),
                                stop=(k == KT - 1),
                            )
                        nc.vector.tensor_copy(
                            out=vp[:sn, n * 8 : (n + 1) * 8, 0:D],
                            in_=ps[:sn].rearrange("p (h d) -> p h d", d=D),
                        )
                        yield
                    vpads.append(vp)
                vpads_by_b.append(vpads)
            stage[pr] = (qt_sb, kt_sb, vpads_by_b)

        def attn_gen(pr):
            """Attention + LayerNorm for both batches of pair `pr`; yields per
            head so pair pr+1 projection matmuls can fill PE idle gaps."""
            qt_sb, kt_sb, vpads_by_b = stage.pop(pr)
            o_by_b = []
            for bi in range(2):
                b = 2 * pr + bi
                vpads = vpads_by_b[bi]
                o_tiles = [
                    o_pool.tile([128, E], F32, tag=f"o{st}", name=f"o{st}_{b}")
                    for st, _ in enumerate(S_TILES)
                ]
                o_by_b.append(o_tiles)
                ps_o4 = None
                for h in range(H):
                    m, r0 = h // 2, (h % 2) * D
                    # scores.T[j, i], both j-tiles in one psum bank -> one
                    # w-mul and one exp per head. Rows 51:128 of the jt=1
                    # slice hold stale psum garbage that is never read
                    # downstream (PV slices [:jn]).
                    ps_s = pp_s.tile([128, 2, S], F32, tag="s", name=f"pss_{b}_{h}")
                    nc.vector.memset(ps_s[:, 1], 0.0)
                    for jt, (js, jn) in enumerate(S_TILES):
                        nc.tensor.matmul(
                            out=ps_s[:jn, jt],
                            lhsT=kt_sb[r0 : r0 + D, m, bi, js : js + jn],
                            rhs=qt_sb[r0 : r0 + D, m, bi, :],
                            start=True,
                            stop=True,
                        )
                    nc.vector.tensor_mul(out=ps_s, in0=ps_s, in1=wsc_sb)
                    p_t = p_pool.tile([128, 2, S], BF16, tag="p", name=f"p_{b}_{h}")
                    nc.scalar.activation(out=p_t, in_=ps_s, func=AF.Exp)

                    # PV: 4 heads share a psum bank: [i, 4, 65] where col 64 of
                    # each head is the softmax denominator (ones column in V).
                    hc = h % 4
                    if hc == 0:
                        ps_o4 = [
                            pp_o.tile([128, 4, D + 1], F32, tag="po", name=f"pso_{b}_{h}_{it}")
                            for it, _ in enumerate(S_TILES)
                        ]
                    for it, (is_, in_n) in enumerate(S_TILES):
                        for jt, (js, jn) in enumerate(S_TILES):
                            nc.tensor.matmul(
                                out=ps_o4[it][:in_n, hc],
                                lhsT=p_t[:jn, jt, is_ : is_ + in_n],
                                rhs=vpads[jt][:jn, h],
                                start=(jt == 0),
                                stop=(jt == 1),
                            )
                    if hc == 3:
                        # Batched normalize for the 4-head group: one
                        # reciprocal of the 4 denominators, one broadcast
                        # multiply writing [i, 4*64] of the output tile.
                        g0 = (h - 3) * D
                        for it, (is_, in_n) in enumerate(S_TILES):
                            rec = r_pool.tile([128, 4], F32, tag="rec4", name=f"rc_{b}_{h}_{it}")
                            nc.vector.reciprocal(
                                out=rec[:in_n], in_=ps_o4[it][:in_n, :, D]
                            )
                            for c in range(4):
                                nc.vector.tensor_scalar_mul(
                                    out=o_tiles[it][:in_n, g0 + c * D : g0 + (c + 1) * D],
                                    in0=ps_o4[it][:in_n, c, 0:D],
                                    scalar1=rec[:in_n, c : c + 1],
                                )
                    yield

            # LayerNorm for both batches last: keeps the ACT table warm (all
            # Exp during attention, then all Sqrt).
            for bi in range(2):
                b = 2 * pr + bi
                for it, (is_, in_n) in enumerate(S_TILES):
                    o_sb = o_by_b[bi][it]
                    stats = ln_pool.tile([128, 2, 6], F32, tag="stats", name=f"st_{b}_{it}")
                    mv = ln_pool.tile([128, 2], F32, tag="mv", name=f"mv_{b}_{it}")
                    nc.vector.bn_stats(out=stats[:in_n, 0], in_=o_sb[:in_n, 0:512])
                    nc.vector.bn_stats(out=stats[:in_n, 1], in_=o_sb[:in_n, 512:E])
                    nc.vector.bn_aggr(out=mv[:in_n], in_=stats[:in_n])
                    rstd = r_pool.tile([128, 1], F32, tag="rstd", name=f"rs_{b}_{it}")
                    nc.scalar.activation(
                        out=rstd[:in_n], in_=mv[:in_n, 1:2], func=AF.Sqrt, bias=eps_t[:in_n]
                    )
                    rrstd = r_pool.tile([128, 1], F32, tag="rrstd", name=f"rr_{b}_{it}")
                    nc.vector.reciprocal(out=rrstd[:in_n], in_=rstd[:in_n])
                    nc.vector.tensor_scalar(
                        out=o_sb[:in_n],
                        in0=o_sb[:in_n],
                        scalar1=mv[:in_n, 0:1],
                        scalar2=rrstd[:in_n],
                        op0=ALU.subtract,
                        op1=ALU.mult,
                    )
                    if apply_gb:
                        nc.vector.tensor_mul(out=o_sb[:in_n], in0=o_sb[:in_n], in1=gamma_b[:in_n])
                        nc.vector.tensor_add(out=o_sb[:in_n], in0=o_sb[:in_n], in1=beta_b[:in_n])
                    nc.sync.dma_start(out=out[b, is_ : is_ + in_n], in_=o_sb[:in_n])
                yield

        # Software pipeline: attention(p) interleaved with projections of later
        # pairs (depth 2) so the PE instruction stream never idles long enough
        # to re-throttle the HAM clock gate — including at pair boundaries.
        from collections import deque

        N_ATTN_CHUNKS = 2 * H + 2    # 34 yields per attn_gen
        for _ in proj_gen(0):
            pass
        pending: deque = deque()
        next_pair = 1

        def push_next():
            nonlocal next_pair
            if next_pair < npair:
                pending.append(proj_gen(next_pair))
                next_pair += 1

        def advance_one():
            while pending:
                if next(pending[0], "END") == "END":
                    pending.popleft()
                    push_next()
                    continue
                return True
            return False

        push_next()
        for p in range(npair):
            ag = attn_gen(p)
            acc = 0
            for _ in ag:
                # Overdrive slightly (28/34 vs the 24 chunks of one proj_gen)
                # so proj(p+1) finishes early and proj(p+2) starts before the
                # pair boundary.
                acc += 28
                while acc >= N_ATTN_CHUNKS:
                    if not advance_one():
                        break
                    acc -= N_ATTN_CHUNKS
            # Boundary: proj(p+1) must be fully emitted before attention(p+1).
            while (p + 1) < npair and (p + 1) not in stage:
                if not advance_one():
                    break


_NC_CACHE: dict = {}


def _get_nc(bpc: int = BPC, apply_gb: bool = True) -> bass.Bass:
    key = (bpc, apply_gb)
    if key not in _NC_CACHE:
        _NC_CACHE[key] = _build_kernel(bpc, apply_gb)
    return _NC_CACHE[key]


def _host_inputs(x, Wq, Wk, Wv, gamma, beta):
    import ml_dtypes

    bf16 = ml_dtypes.bfloat16
    x = np.asarray(x, dtype=np.float32)
    xT = np.ascontiguousarray(x.transpose(0, 2, 1)).astype(bf16)  # [B, E, S]
    idx = np.arange(S, dtype=np.float32)
    wsc = (np.abs(idx[None, :] - idx[:, None]) / S * SCALE).astype(np.float32)
    common = {
        "wqT": np.ascontiguousarray(np.asarray(Wq, np.float32).T).astype(bf16),
        "wkT": np.ascontiguousarray(np.asarray(Wk, np.float32).T).astype(bf16),
        "wvT": np.ascontiguousarray(np.asarray(Wv, np.float32).T).astype(bf16),
        "wsc": wsc,
        "gamma": np.asarray(gamma, np.float32),
        "beta": np.asarray(beta, np.float32),
    }
    return xT, common


def run(inputs: dict, trace: bool = False, trace_dir: str | None = None):
    """Run the SPMD kernel on 8 cores. Returns (full_output, exec_time_ns)."""
    xT, common = _host_inputs(**inputs)
    in_maps = [
        {**common, "xT": np.ascontiguousarray(xT[c * BPC : (c + 1) * BPC])}
        for c in range(NCORES)
    ]
    apply_gb = not (
        np.all(np.asarray(inputs["gamma"]) == 1.0)
        and np.all(np.asarray(inputs["beta"]) == 0.0)
    )
    nc = _get_nc(BPC, apply_gb)
    res = run_bass_kernel_spmd(
        nc, in_maps, core_ids=list(range(NCORES)), trace=trace, tmpdir=trace_dir
    )
    full = np.concatenate([res.results[c]["out"] for c in range(NCORES)], axis=0)
    return full.astype(np.float32), res.exec_time_ns


def kernel(x, Wq, Wk, Wv, gamma, beta):
    full, _ = run(dict(x=x, Wq=Wq, Wk=Wk, Wv=Wv, gamma=gamma, beta=beta))
    return full


# revision 26
# speedup vs baseline: 1.4788x; 1.1652x over previous
"""Trainium2 Bass kernel for nn_Attention_6468220748045.

Computes, per batch item: QKV projection -> per-head scaled attention with a
multiplicative positional bias w[i,j] = |i-j|/S -> softmax -> attn @ V ->
LayerNorm over the embedding dim.

Sharding: pure data-parallel over batch. B=128 splits as 16 batch items per
core across 8 NeuronCores; no collectives needed. Inputs are pre-laid-out on
host: x is passed transposed per batch ([B, E, S]) so both projection
orientations stream directly from SBUF, and the weights are passed transposed
([e_in, e_out]) to serve as matmul stationary operands.

Per-core kernel layout choices:
  - QT/KT projections: stationary = W.T tile [e_in,128 x e_out,128], moving =
    x.T for a PAIR of batches ([e_in,128 x 358]) -> fp32r runs at full rate
    (moving dim >= 256). Output orientation [e_out, s] is exactly what the
    scores matmul needs (contraction over head dim on the partition axis).
  - V projection: stationary = x.T tile, moving = Wv.T ([e_in,128 x 512]),
    giving V in natural [s, e] orientation for the PV matmul.
  - Scores: s.T[j, i] = (k_h).T.T @ (q_h).T in bf16, multiplied by the
    host-precomputed scale*w[j, i], exponentiated on ScalarE (no max
    subtraction: |scores| <= ~2, exp is safe).
  - Softmax denominator comes for free from the PV matmul: V is stored padded
    [s, H, 65] with a ones column, so out[:, 64] = sum_j p[j, i].
  - PV: stationary = p.T tile (bf16), moving = padded V tile; normalize by the
    reciprocal of the ones-column and write straight into the [s, e] output
    tile, which then gets LayerNorm'd (bn_stats/bn_aggr) and DMA'd out.
"""

import numpy as np

import concourse.bass as bass
import concourse.tile as tile
from concourse import bacc, mybir
from concourse.bass_utils import run_bass_kernel_spmd

# Problem constants (hardcoded per the self-contained-kernel contract).
B, S, E, H, D = 128, 179, 1024, 16, 64
NCORES = 8
BPC = B // NCORES          # batches per core = 16
NPAIR = BPC // 2           # batch pairs per core = 8
KT = E // 128              # contraction tiles over e_in = 8
MT = E // 128              # output tiles over e_out = 8
S0 = 128                   # first s-tile size
S1 = S - S0                # second s-tile size = 51
S_TILES = ((0, S0), (S0, S1))
LN_EPS = 1e-5
SCALE = float(E) ** -0.5

F32 = mybir.dt.float32
F32R = mybir.dt.float32r
BF16 = mybir.dt.bfloat16

AF = mybir.ActivationFunctionType
ALU = mybir.AluOpType


def _build_kernel(bpc: int = BPC, apply_gb: bool = True) -> bass.Bass:
    npair = bpc // 2
    nc = bacc.Bacc()

    xT = nc.dram_tensor("xT", [bpc, E, S], BF16, kind="ExternalInput").ap()
    wqT = nc.dram_tensor("wqT", [E, E], BF16, kind="ExternalInput").ap()
    wkT = nc.dram_tensor("wkT", [E, E], BF16, kind="ExternalInput").ap()
    wvT = nc.dram_tensor("wvT", [E, E], BF16, kind="ExternalInput").ap()
    wsc = nc.dram_tensor("wsc", [S, S], F32, kind="ExternalInput").ap()
    gamma = nc.dram_tensor("gamma", [E], F32, kind="ExternalInput").ap()
    beta = nc.dram_tensor("beta", [E], F32, kind="ExternalInput").ap()
    out = nc.dram_tensor("out", [bpc, S, E], F32, kind="ExternalOutput").ap()

    with tile.TileContext(nc) as tc:
        _emit(tc, npair, out, xT, wqT, wkT, wvT, wsc, gamma, beta, apply_gb)
    nc.compile()
    return nc


def _emit(tc, npair, out, xT, wqT, wkT, wvT, wsc, gamma, beta, apply_gb):
    nc = tc.nc
    from contextlib import ExitStack

    with ExitStack() as ctx:
        singles = ctx.enter_context(tc.tile_pool(name="singles", bufs=1))
        xt_pool = ctx.enter_context(tc.tile_pool(name="xt", bufs=3))
        qk_pool = ctx.enter_context(tc.tile_pool(name="qk", bufs=3))
        v_pool = ctx.enter_context(tc.tile_pool(name="v", bufs=6))
        p_pool = ctx.enter_context(tc.tile_pool(name="p", bufs=4))
        o_pool = ctx.enter_context(tc.tile_pool(name="o", bufs=3))
        ln_pool = ctx.enter_context(tc.tile_pool(name="ln", bufs=4))
        r_pool = ctx.enter_context(tc.tile_pool(name="r", bufs=8))

        pp_qk = ctx.enter_context(tc.tile_pool(name="pp_qk", bufs=2, space="PSUM"))
        pp_v = ctx.enter_context(tc.tile_pool(name="pp_v", bufs=2, space="PSUM"))
        pp_s = ctx.enter_context(tc.tile_pool(name="pp_s", bufs=2, space="PSUM"))
        pp_o = ctx.enter_context(tc.tile_pool(name="pp_o", bufs=2, space="PSUM"))

        # --- resident tensors -------------------------------------------------
        # Weight tiles: [e_in partition, k-tile, e_out]. DMA order matters for
        # startup latency: wq first, then pair-0's x.T, then wk/wv — the first
        # Q.T matmuls only need wq + x.T.
        xsrc = xT.rearrange("b (k p) s -> k p b s", p=128)  # [KT, 128, bpc, S]
        w_sbs = []
        for name, wap in (("wq", wqT), ("wk", wkT), ("wv", wvT)):
            w_sb = singles.tile([128, KT, E], BF16, tag=f"w_{name}")
            w_sbs.append(w_sb)
        wq_sb, wk_sb, wv_sb = w_sbs
        xt0 = xt_pool.tile([128, KT, 2, S], BF16, tag="xt", name="xt_0")
        for w_sb, wap in ((wq_sb, wqT),):
            src = wap.rearrange("(k p) e -> k p e", p=128)
            for k in range(KT):
                nc.sync.dma_start(out=w_sb[:, k], in_=src[k])
        for k in range(KT):
            nc.sync.dma_start(out=xt0[:, k], in_=xsrc[k, :, 0:2, :])
        for w_sb, wap in ((wk_sb, wkT), (wv_sb, wvT)):
            src = wap.rearrange("(k p) e -> k p e", p=128)
            for k in range(KT):
                nc.sync.dma_start(out=w_sb[:, k], in_=src[k])

        # Positional bias (already includes softmax scale): [j partition, jt, i]
        wsc_sb = singles.tile([128, 2, S], F32, tag="wsc")
        nc.vector.memset(wsc_sb[:, 1], 0.0)
        nc.sync.dma_start(out=wsc_sb[:, 0], in_=wsc[0:S0])
        nc.sync.dma_start(out=wsc_sb[0:S1, 1], in_=wsc[S0:S])

        # gamma/beta broadcast to all partitions (skipped when the caller
        # verified they are identity); eps scalar.
        if apply_gb:
            gamma_b = singles.tile([128, E], F32, tag="gamma")
            beta_b = singles.tile([128, E], F32, tag="beta")
            nc.sync.dma_start(
                out=gamma_b,
                in_=bass.AP(tensor=gamma.tensor, offset=gamma.offset, ap=[[0, 128]] + gamma.ap),
            )
            nc.sync.dma_start(
                out=beta_b,
                in_=bass.AP(tensor=beta.tensor, offset=beta.offset, ap=[[0, 128]] + beta.ap),
            )
        eps_t = singles.tile([128, 1], F32, tag="eps")
        nc.vector.memset(eps_t, LN_EPS)

        # Per-pair SBUF products handed from the projection stage to the
        # attention stage (software pipeline).
        stage: dict = {}

        def proj_gen(pr):
            """QKV projections for batch pair `pr`; yields after each PE chunk
            (~8 matmuls) so attention of pair pr-1 can be interleaved."""
            if pr == 0:
                xt = xt0
            else:
                xt = xt_pool.tile([128, KT, 2, S], BF16, tag="xt", name=f"xt_{pr}")
                for k in range(KT):
                    nc.sync.dma_start(
                        out=xt[:, k], in_=xsrc[k, :, 2 * pr : 2 * pr + 2, :]
                    )

            # Q.T / K.T: out[e_out, s2], s2 = 2*S = 358 (both batches at once).
            # kt is stored s-padded to 2*128 per batch with a zeroed tail, so
            # the jt=1 scores matmul has a full 128-wide stationary and writes
            # all 128 psum rows (zeros beyond row 51) — no per-head memset.
            qt_sb = qk_pool.tile([128, MT, 2, S], BF16, tag="qt", name=f"qt_{pr}")
            kt_sb = qk_pool.tile([128, MT, 2, 2 * 128], BF16, tag="kt", name=f"kt_{pr}")
            nc.vector.memset(kt_sb[:, :, :, S:], 0.0)
            for w_sb, dst, pad in ((wq_sb, qt_sb, False), (wk_sb, kt_sb, True)):
                for m in range(MT):
                    ps = pp_qk.tile([128, 2, S], F32, tag="qk", name=f"psqk_{pr}_{m}")
                    for k in range(KT):
                        nc.tensor.matmul(
                            out=ps,
                            lhsT=w_sb[:, k, m * 128 : (m + 1) * 128],
                            rhs=xt[:, k],
                            start=(k == 0),
                            stop=(k == KT - 1),
                        )
                    if pad:
                        nc.vector.tensor_copy(out=dst[:, m, :, 0:S], in_=ps)
                    else:
                        nc.vector.tensor_copy(out=dst[:, m], in_=ps)
                    yield

            # V: natural [s, e] layout with a ones column appended per head
            vpads_by_b = []
            for bi in range(2):
                vpads = []
                for st, (ss, sn) in enumerate(S_TILES):
                    vp = v_pool.tile(
                        [128, H, D + 1], BF16, tag=f"vpad{st}", name=f"vp{st}_{pr}_{bi}"
                    )
                    nc.vector.memset(vp[:sn, :, D : D + 1], 1.0)
                    for n in range(2):
                        ps = pp_v.tile([128, 512], F32, tag="v", name=f"psv_{pr}_{bi}_{st}_{n}")
                        for k in range(KT):
                            nc.tensor.matmul(
                                out=ps[:sn],
                                lhsT=xt[:, k, bi, ss : ss + sn],
                                rhs=wv_sb[:, k, n * 512 : (n + 1) * 512],
                                start=(k == 0),
                                stop=(k == KT - 1),
                            )
                        nc.vector.tensor_copy(
                            out=vp[:sn, n * 8 : (n + 1) * 8, 0:D],
                            in_=ps[:sn].rearrange("p (h d) -> p h d", d=D),
                        )
                        yield
                    vpads.append(vp)
                vpads_by_b.append(vpads)
            stage[pr] = (qt_sb, kt_sb, vpads_by_b)

        def attn_gen(pr):
            """Attention + LayerNorm for both batches of pair `pr`; yields per
            head so pair pr+1 projection matmuls can fill PE idle gaps."""
            qt_sb, kt_sb, vpads_by_b = stage.pop(pr)
            o_by_b = []
            for bi in range(2):
                b = 2 * pr + bi
                vpads = vpads_by_b[bi]
                o_tiles = [
                    o_pool.tile([128, E], F32, tag=f"o{st}", name=f"o{st}_{b}")
                    for st, _ in enumerate(S_TILES)
                ]
                o_by_b.append(o_tiles)
                ps_o4 = None
                for h in range(H):
                    m, r0 = h // 2, (h % 2) * D
                    # scores.T[j, i], both j-tiles in one psum bank -> one
                    # w-mul and one exp per head. The jt=1 stationary is the
                    # zero-padded 128-wide kt slice, so all psum rows are
                    # written (zeros beyond row 51).
                    ps_s = pp_s.tile([128, 2, S], F32, tag="s", name=f"pss_{b}_{h}")
                    for jt in range(2):
                        nc.tensor.matmul(
                            out=ps_s[:, jt],
                            lhsT=kt_sb[r0 : r0 + D, m, bi, jt * 128 : (jt + 1) * 128],
                            rhs=qt_sb[r0 : r0 + D, m, bi, :],
                            start=True,
                            stop=True,
                        )
                    nc.vector.tensor_mul(out=ps_s, in0=ps_s, in1=wsc_sb)
                    p_t = p_pool.tile([128, 2, S], BF16, tag="p", name=f"p_{b}_{h}")
                    nc.scalar.activation(out=p_t, in_=ps_s, func=AF.Exp)

                    # PV: 4 heads share a psum bank: [i, 4, 65] where col 64 of
                    # each head is the softmax denominator (ones column in V).
                    hc = h % 4
                    if hc == 0:
                        ps_o4 = [
                            pp_o.tile([128, 4, D + 1], F32, tag="po", name=f"pso_{b}_{h}_{it}")
                            for it, _ in enumerate(S_TILES)
                        ]
                    for it, (is_, in_n) in enumerate(S_TILES):
                        for jt, (js, jn) in enumerate(S_TILES):
                            nc.tensor.matmul(
                                out=ps_o4[it][:in_n, hc],
                                lhsT=p_t[:jn, jt, is_ : is_ + in_n],
                                rhs=vpads[jt][:jn, h],
                                start=(jt == 0),
                                stop=(jt == 1),
                            )
                    if hc == 3:
                        # Batched normalize for the 4-head group: one
                        # reciprocal of the 4 denominators, one broadcast
                        # multiply writing [i, 4*64] of the output tile.
                        g0 = (h - 3) * D
                        for it, (is_, in_n) in enumerate(S_TILES):
                            rec = r_pool.tile([128, 4], F32, tag="rec4", name=f"rc_{b}_{h}_{it}")
                            nc.vector.reciprocal(
                                out=rec[:in_n], in_=ps_o4[it][:in_n, :, D]
                            )
                            for c in range(4):
                                nc.vector.tensor_scalar_mul(
                                    out=o_tiles[it][:in_n, g0 + c * D : g0 + (c + 1) * D],
                                    in0=ps_o4[it][:in_n, c, 0:D],
                                    scalar1=rec[:in_n, c : c + 1],
                                )
                    yield "h"

            # LayerNorm for both batches last: keeps the ACT table warm (all
            # Exp during attention, then all Sqrt).
            for bi in range(2):
                b = 2 * pr + bi
                for it, (is_, in_n) in enumerate(S_TILES):
                    o_sb = o_by_b[bi][it]
                    stats = ln_pool.tile([128, 2, 6], F32, tag="stats", name=f"st_{b}_{it}")
                    mv = ln_pool.tile([128, 2], F32, tag="mv", name=f"mv_{b}_{it}")
                    nc.vector.bn_stats(out=stats[:in_n, 0], in_=o_sb[:in_n, 0:512])
                    nc.vector.bn_stats(out=stats[:in_n, 1], in_=o_sb[:in_n, 512:E])
                    nc.vector.bn_aggr(out=mv[:in_n], in_=stats[:in_n])
                    rstd = r_pool.tile([128, 1], F32, tag="rstd", name=f"rs_{b}_{it}")
                    nc.scalar.activation(
                        out=rstd[:in_n], in_=mv[:in_n, 1:2], func=AF.Sqrt, bias=eps_t[:in_n]
                    )
                    rrstd = r_pool.tile([128, 1], F32, tag="rrstd", name=f"rr_{b}_{it}")
                    nc.vector.reciprocal(out=rrstd[:in_n], in_=rstd[:in_n])
                    nc.vector.tensor_scalar(
                        out=o_sb[:in_n],
                        in0=o_sb[:in_n],
                        scalar1=mv[:in_n, 0:1],
                        scalar2=rrstd[:in_n],
                        op0=ALU.subtract,
                        op1=ALU.mult,
                    )
                    if apply_gb:
                        nc.vector.tensor_mul(out=o_sb[:in_n], in0=o_sb[:in_n], in1=gamma_b[:in_n])
                        nc.vector.tensor_add(out=o_sb[:in_n], in0=o_sb[:in_n], in1=beta_b[:in_n])
                    nc.sync.dma_start(out=out[b, is_ : is_ + in_n], in_=o_sb[:in_n])
                yield "ln"

        # Software pipeline: attention(p) interleaved with projections of later
        # pairs (depth 2) so the PE instruction stream never idles long enough
        # to re-throttle the HAM clock gate — including at pair boundaries.
        from collections import deque

        N_ATTN_CHUNKS = 2 * H + 2    # 34 yields per attn_gen
        for _ in proj_gen(0):
            pass
        pending: deque = deque()
        next_pair = 1

        def push_next():
            nonlocal next_pair
            if next_pair < npair:
                pending.append(proj_gen(next_pair))
                next_pair += 1

        def advance_one():
            while pending:
                if next(pending[0], "END") == "END":
                    pending.popleft()
                    push_next()
                    continue
                return True
            return False

        push_next()
        for p in range(npair):
            ag = attn_gen(p)
            acc = 0
            for tok in ag:
                # Distribute ~24 proj chunks per pair: 18 across the 32 head
                # yields, 3 at each LN yield (where attention gives the PE the
                # least work).
                if tok == "ln":
                    for _ in range(3):
                        if not advance_one():
                            break
                else:
                    acc += 18
                    while acc >= 32:
                        if not advance_one():
                            break
                        acc -= 32
            # Boundary: proj(p+1) must be fully emitted before attention(p+1).
            while (p + 1) < npair and (p + 1) not in stage:
                if not advance_one():
                    break


_NC_CACHE: dict = {}


def _get_nc(bpc: int = BPC, apply_gb: bool = True) -> bass.Bass:
    key = (bpc, apply_gb)
    if key not in _NC_CACHE:
        _NC_CACHE[key] = _build_kernel(bpc, apply_gb)
    return _NC_CACHE[key]


def _host_inputs(x, Wq, Wk, Wv, gamma, beta):
    import ml_dtypes

    bf16 = ml_dtypes.bfloat16
    x = np.asarray(x, dtype=np.float32)
    xT = np.ascontiguousarray(x.transpose(0, 2, 1)).astype(bf16)  # [B, E, S]
    idx = np.arange(S, dtype=np.float32)
    wsc = (np.abs(idx[None, :] - idx[:, None]) / S * SCALE).astype(np.float32)
    common = {
        "wqT": np.ascontiguousarray(np.asarray(Wq, np.float32).T).astype(bf16),
        "wkT": np.ascontiguousarray(np.asarray(Wk, np.float32).T).astype(bf16),
        "wvT": np.ascontiguousarray(np.asarray(Wv, np.float32).T).astype(bf16),
        "wsc": wsc,
        "gamma": np.asarray(gamma, np.float32),
        "beta": np.asarray(beta, np.float32),
    }
    return xT, common


def run(inputs: dict, trace: bool = False, trace_dir: str | None = None):
    """Run the SPMD kernel on 8 cores. Returns (full_output, exec_time_ns)."""
    xT, common = _host_inputs(**inputs)
    in_maps = [
        {**common, "xT": np.ascontiguousarray(xT[c * BPC : (c + 1) * BPC])}
        for c in range(NCORES)
    ]
    apply_gb = not (
        np.all(np.asarray(inputs["gamma"]) == 1.0)
        and np.all(np.asarray(inputs["beta"]) == 0.0)
    )
    nc = _get_nc(BPC, apply_gb)
    res = run_bass_kernel_spmd(
        nc, in_maps, core_ids=list(range(NCORES)), trace=trace, tmpdir=trace_dir
    )
    full = np.concatenate([res.results[c]["out"] for c in range(NCORES)], axis=0)
    return full.astype(np.float32), res.exec_time_ns


def kernel(x, Wq, Wk, Wv, gamma, beta):
    full, _ = run(dict(x=x, Wq=Wq, Wk=Wk, Wv=Wv, gamma=gamma, beta=beta))
    return full
